# revision 30
# baseline (speedup 1.0000x reference)
"""Trainium2 Bass kernel for nn_Encoder (S=4096, D=512, H=8, E=64).

Sharding: sequence-parallel over 8 cores. Each core computes the full K/V
(every query needs them) plus attention/MLP for its own 512 rows; the only
cross-core traffic is two 8-byte AllReduces for the global LayerNorm
statistics (the reference normalizes jointly over the whole [S, D] tensor).
The host concatenates the per-core row shards.

v3 (all-SBUF, interleaved): K^T and V' live entirely in SBUF (no DRAM
scratch, no DMA in the attention inner loop). x^T is produced by PE
transposes fed from plain row-major x loads (the DMA-transpose path was the
original phase-1 bottleneck). Per-core dataflow:
  - x rows stream in fp32 on the sync queue and are transposed directly on
    the PE (4 tiles per PSUM bank, one grouped ACT evacuation per bank that
    also casts fp32->bf16).
  - K^T is built pair-packed [(h%2)*64+e, pair, t] so logits for both heads
    of a pair contract at partition ranges 0:64 / 64:128 with no padding.
  - V' [t%128, chunk, h, e'] carries a ones column (e'=64) so the softmax
    denominator falls out of AV row 64.
  - logits are computed transposed, L^T[t, q], so the Exp output is already
    the A@V moving operand; AV is skewed one chunk behind exp. Attention
    pass 0 is interleaved into phase 1 (one chunk-group behind), soaking up
    phase-1 ACT slack; passes 1-3 follow, ACT(exp)/power-bound.
  - outH^T is re-packed into head pairs (h1 halves moved to partitions
    64:128 by an SBUF-to-SBUF DMA) so the output projection contracts both
    heads of a pair per matmul against the pair-packed Wo.
  - collectives: mesh AllReduces are ~10us warm but 30-50us cold, so a
    dummy 8B AllReduce runs at startup and after passes 1/2 (data-dependent
    on the pass output so the scheduler cannot hoist it); the LN1 window is
    filled with the Kp/Vp own-projections, Vp's output projection rides the
    MLP, and W1/W2 stream in during attention by reusing x^T tile slots.
  - weight loads are spread across all three DMA paths (Wq/Wk fp32-staged
    on scalar/sync HWDGE + DVE cast, Wv on the gpsimd converting path) so
    no single serial queue gates the first matmuls.
"""

import os

os.environ.setdefault("JAX_PLATFORMS", "axon")

import numpy as np

import concourse.bass as bass
import concourse.tile as tile
from concourse import mybir
from concourse.bass_utils import run_bass_kernel_spmd
from concourse.masks import make_identity

dt = mybir.dt
AF = mybir.ActivationFunctionType
ALU = mybir.AluOpType
AX = mybir.AxisListType

N_CORES = 8
S, D, H, E = 4096, 512, 8, 64
F = 4 * D          # 2048
R = S // N_CORES   # 512 rows per core
EPS = 1e-5
SCALE = 1.0 / float(np.sqrt(E))
INV_SD = 1.0 / float(S * D)


def split_waits(nc):
    """Walrus codegen allows only one sync-wait per HW instruction. Move
    extra waits onto single-wait NoOps inserted before, same engine queue."""
    import bass_rust

    n = 0
    for bb in nc.m.functions[0].blocks:
        new_list = []
        changed = False
        for ins in bb.instructions:
            si = ins.sync_info
            if si is not None and si.on_wait is not None and len(si.on_wait) > 1:
                waits = list(si.on_wait)
                for w in waits[:-1]:
                    nop = bass_rust.InstNoOp(name=f"I-xwait-{n}")
                    n += 1
                    nop.engine = ins.engine
                    nop.sync_info = bass_rust.SyncInfo(on_wait=[w], on_update=[])
                    nc.register_instruction(nop)
                    new_list.append(nop)
                si.on_wait = waits[-1:]
                ins.sync_info = si
                changed = True
            new_list.append(ins)
        if changed:
            bb.instructions = new_list
    return nc


def build_nc():
    import contextlib

    nc = bass.Bass("TRN2", debug=False, num_devices=N_CORES)
    f32, f32r = dt.float32, dt.float32r
    bf16 = dt.bfloat16

    # ---- I/O ----------------------------------------------------------
    x_d = nc.dram_tensor("x", [S, D], f32, kind="ExternalInput").ap()
    Wq_d = nc.dram_tensor("Wq", [H, D, E], f32, kind="ExternalInput").ap()
    Wk_d = nc.dram_tensor("Wk", [H, D, E], f32, kind="ExternalInput").ap()
    Wv_d = nc.dram_tensor("Wv", [H, D, E], f32, kind="ExternalInput").ap()
    bq_d = nc.dram_tensor("bq", [H, E], f32, kind="ExternalInput").ap()
    bk_d = nc.dram_tensor("bk", [H, E], f32, kind="ExternalInput").ap()
    bv_d = nc.dram_tensor("bv", [H, E], f32, kind="ExternalInput").ap()
    Wo_d = nc.dram_tensor("Wo", [D, D], f32, kind="ExternalInput").ap()
    bo_d = nc.dram_tensor("bo", [D], f32, kind="ExternalInput").ap()
    W1_d = nc.dram_tensor("W1", [D, F], f32, kind="ExternalInput").ap()
    b1_d = nc.dram_tensor("b1", [F], f32, kind="ExternalInput").ap()
    W2_d = nc.dram_tensor("W2", [F, D], f32, kind="ExternalInput").ap()
    b2_d = nc.dram_tensor("b2", [D], f32, kind="ExternalInput").ap()
    xr_d = nc.dram_tensor("x_rows", [R, D], f32, kind="ExternalInput").ap()
    lng_d = nc.dram_tensor("ln_g_rows", [R, D], f32, kind="ExternalInput").ap()
    lnb_d = nc.dram_tensor("ln_b_rows", [R, D], f32, kind="ExternalInput").ap()

    fin_d = nc.dram_tensor("final_rows", [R, D], f32, kind="ExternalOutput").ap()
    kp_d = nc.dram_tensor("Kp_rows", [R, D], f32, kind="ExternalOutput").ap()
    vp_d = nc.dram_tensor("Vp_rows", [R, D], f32, kind="ExternalOutput").ap()

    # row index q = qc*128 + p everywhere
    x_v = x_d.rearrange("(tt c p) d -> tt p c d", p=128, c=4)
    xr_v = xr_d.rearrange("(c p) d -> p c d", p=128)
    lng_v = lng_d.rearrange("(c p) d -> p c d", p=128)
    lnb_v = lnb_d.rearrange("(c p) d -> p c d", p=128)
    fin_v = fin_d.rearrange("(c p) d -> p c d", p=128)
    kp_v = kp_d.rearrange("(c p) d -> p c d", p=128)
    vp_v = vp_d.rearrange("(c p) d -> p c d", p=128)

    NT = S // 512  # 8 row chunks of 512

    with tile.TileContext(nc) as tc, contextlib.ExitStack() as ctx, \
            nc.allow_low_precision(reason="bf16 matmul operands, fp32 accumulate"):
        ep = ctx.enter_context

        # ---- pools ----------------------------------------------------
        single = ep(tc.tile_pool(name="single", bufs=1))
        kt_p = ep(tc.tile_pool(name="kt", bufs=1))        # K^T resident
        vp_p = ep(tc.tile_pool(name="vp", bufs=1))        # V' resident
        xin_p = ep(tc.tile_pool(name="xin", bufs=2))      # x fp32 in
        stg_p = ep(tc.tile_pool(name="stg", bufs=2))      # w staging / sq / h1T
        xt_p = ep(tc.tile_pool(name="xt", bufs=6))        # x^T bf16 / W1 / W2
        c8 = ep(tc.tile_pool(name="c8", bufs=2))          # xro(z), out1(w), fin
        qt_p = ep(tc.tile_pool(name="qt", bufs=1))        # Q^T pair-packed
        ot_p = ep(tc.tile_pool(name="ot", bufs=1))        # outH^T pair-packed
        pexp_p = ep(tc.tile_pool(name="pexp", bufs=3))
        otr_p = ep(tc.tile_pool(name="otr", bufs=2))
        otr1 = ep(tc.tile_pool(name="otr1", bufs=1))      # rden / oth
        d16 = ep(tc.tile_pool(name="d16", bufs=1))        # KTo
        evac = ep(tc.tile_pool(name="evac", bufs=2))
        wk = ep(tc.tile_pool(name="wk", bufs=2))
        # psum: tag "big" 3x2banks + tag "po" 2x1bank = 8 banks
        ps_big = ep(tc.tile_pool(name="ps_big", bufs=3, space="PSUM"))
        ps_po = ep(tc.tile_pool(name="ps_po", bufs=2, space="PSUM"))
        dram = ep(tc.tile_pool(name="dram", bufs=1, space="DRAM"))

        # ---- constants / small loads ---------------------------------
        ident = single.tile([128, 128], f32)
        make_identity(nc, ident[:])
        onesP = single.tile([128, 8], f32)
        nc.vector.memset(onesP[:], 1.0)
        ones_row = single.tile([1, 128], bf16)
        nc.vector.memset(ones_row[:], 1.0)
        ones_row_r = single.tile([1, 128], f32r)
        nc.vector.tensor_copy(ones_row_r[:], ones_row[:])
        eps_t = single.tile([1, 1], f32)
        nc.vector.memset(eps_t[:], EPS)

        # Dummy 8-byte AllReduce: absorbs the first-collective mesh setup
        # (~50us observed) in the shadow of phase 1 + attention, so the real
        # LN AllReduces behave like steady-state (~10us). Result unread.
        warm = wk.tile([1, 2], f32, tag="warm")
        nc.vector.memset(warm[:], 0.0)
        cin0 = dram.tile([1, 2], f32)
        cout0 = dram.tile([1, 2], f32)
        nc.sync.dma_start(cin0[:], warm[:])
        nc.gpsimd.collective_compute(
            "AllReduce", ALU.add,
            replica_groups=[list(range(N_CORES))],
            ins=[cin0[:]], outs=[cout0[:]],
        )

        # x own rows fp32 first on the scalar queue (feeds xrT -> Q^T)
        xro = c8.tile([128, 4, D], f32, tag="c8")
        nc.scalar.dma_start(xro[:], xr_v)

        def transpose_512(dst_bf16, src_f32):
            """dst[128, 4, 512] bf16 (d-major) = transpose of src[128, 4, 512]
            fp32 (t-major). 16 PE transposes, 4 per PSUM bank, grouped ACT
            evacs that also do the fp32->bf16 cast."""
            for dc in range(4):
                ptr = ps_po.tile([128, 4, 128], f32, tag="po")
                for c in range(4):
                    nc.tensor.transpose(
                        ptr[:, c, :], src_f32[:, c, dc * 128:(dc + 1) * 128],
                        ident[:],
                    )
                nc.scalar.activation(
                    dst_bf16[:, dc, :].rearrange("p (c q) -> p c q", c=4),
                    ptr[:], AF.Identity,
                )

        # own-rows x^T right away (x_rows is an input because the core id is
        # not known at compile time)
        xrT = single.tile([128, 4, R], bf16, tag="xrT")  # read until Kp/Vp
        transpose_512(xrT, xro)

        # First x chunk leads the sync queue (the wk staging DMAs behind it
        # take ~17us and must not gate the first transposes)
        xin_pre = {}
        _x0 = xin_p.tile([128, 4, 512], f32, tag="xin")
        nc.sync.dma_start(_x0[:], x_v[0])
        xin_pre[0] = _x0

        # Wq/Wk/Wv as [p=d%128, dc, he] with he = (h//2)*128 + (h%2)*64 + e.
        # The strided converting loads are ~5us each on the serial gpsimd
        # SWDGE queue and gate the first matmuls — so Wq/Wk load fp32 via
        # the fast HWDGE queues (scalar/sync) into staging + DVE cast, and
        # only Wv (needed latest) takes the gpsimd converting path.
        w_qkv = {}
        for name, wd, eng in (("q", Wq_d, nc.scalar), ("k", Wk_d, nc.sync)):
            stg = stg_p.tile([128, 4, D], f32, tag="stg")
            wv4 = wd.rearrange("h (dc p) e -> dc p h e", p=128)
            for dc in range(4):
                eng.dma_start(
                    stg[:, dc, :].rearrange("p (h e) -> p h e", e=E), wv4[dc]
                )
            t = single.tile([128, 4, D], bf16, tag=f"w{name}")
            nc.vector.tensor_copy(t[:], stg[:])
            w_qkv[name] = t
        # tiny packed-pair biases first on the gpsimd queue (they gate the
        # Q^T / K^T evacuations), then the big Wv converting load
        bqs2 = single.tile([128, 4], f32)
        nc.gpsimd.dma_start(bqs2[:], bq_d.rearrange("(c h2) e -> (h2 e) c", h2=2))
        bks2 = single.tile([128, 4], f32)
        nc.gpsimd.dma_start(bks2[:], bk_d.rearrange("(c h2) e -> (h2 e) c", h2=2))
        wv_t = single.tile([128, 4, D], bf16, tag="wv")
        wv4 = Wv_d.rearrange("h (dc p) e -> dc p h e", p=128)
        for dc in range(4):
            nc.gpsimd.dma_start(
                wv_t[:, dc, :].rearrange("p (h e) -> p h e", e=E), wv4[dc]
            )
        w_qkv["v"] = wv_t

        bvs2 = single.tile([128, 4], f32)
        nc.gpsimd.dma_start(bvs2[:], bv_d.rearrange("(c h2) e -> (h2 e) c", h2=2))
        bv_bc = single.tile([128, D], bf16)
        bv_flat = bv_d.rearrange("h e -> (h e)")
        nc.gpsimd.dma_start(
            bv_bc[:],
            bass.AP(tensor=bv_flat.tensor, offset=bv_flat.offset,
                    ap=[[0, 128]] + [list(a) for a in bv_flat.ap]),
        )

        # Wo packed by head pair: [(h%2)*64+e, h//2, dm] — used by the main
        # output projection (against pair-packed outH^T) and by Kp/Vp.
        Wo_p = single.tile([128, 4, D], bf16)
        nc.gpsimd.dma_start(Wo_p[:], Wo_d.rearrange("(c h2 e) d -> (h2 e) c d", h2=2, e=E))

        b1s = single.tile([128, 16], f32)
        nc.gpsimd.dma_start(b1s[:], b1_d.rearrange("(c p) -> p c", p=128))
        bo_r = single.tile([1, D], bf16)
        b2_r = single.tile([1, D], bf16)
        nc.gpsimd.dma_start(bo_r[:], bo_d.rearrange("(o d) -> o d", o=1))
        nc.gpsimd.dma_start(b2_r[:], b2_d.rearrange("(o d) -> o d", o=1))

        # LN gains/biases for own rows: loaded once (bf16), used by both LNs
        lng_s = single.tile([128, 4, D], bf16)
        lnb_s = single.tile([128, 4, D], bf16)
        nc.gpsimd.dma_start(lng_s[:], lng_v)
        nc.gpsimd.dma_start(lnb_s[:], lnb_v)

        # ---- persistent K^T / V' -------------------------------------
        KT = kt_p.tile([128, 4, S], bf16)              # [(h%2)*64+e, pair, t]
        VP = vp_p.tile([128, 32, H, E + 1], bf16)      # [t%128, chunk, h, e']
        nc.vector.memset(VP[:, :, :, E], 1.0)          # ones column

        # ---- phase 1: x^T via PE transpose -> K^T, V' ----------------
        def own_proj_packed(dst, w_t, bias2_t):
            """dst[128, mc, R] = pair-packed (x_rows @ W)^T + b."""
            for mch in range(2):
                pq = ps_big.tile([128, 2, 512], f32, tag="big")
                for half in range(2):
                    mc = mch * 2 + half
                    for dc in range(4):
                        nc.tensor.matmul(
                            pq[:, half, :],
                            lhsT=w_t[:, dc, mc * 128:(mc + 1) * 128],
                            rhs=xrT[:, dc, :],
                            start=(dc == 0), stop=(dc == 3),
                        )
                    nc.scalar.activation(
                        dst[:, mc, :], pq[:, half, :], AF.Identity,
                        bias=bias2_t[:, mc:mc + 1],
                    )

        def wo_project_packed(src_T, out_view):
            """out_view rows = concat_h(src) @ Wo + bo (src packed [128,4,R])."""
            for qch in range(2):
                po = ps_big.tile([128, 2, 512], f32, tag="big")
                for half in range(2):
                    qc = qch * 2 + half
                    for mc in range(4):
                        nc.tensor.matmul(
                            po[:, half, :],
                            lhsT=src_T[:, mc, qc * 128:(qc + 1) * 128],
                            rhs=Wo_p[:, mc, :],
                            start=(mc == 0), stop=False,
                        )
                    nc.tensor.matmul(
                        po[:, half, :], lhsT=ones_row[:], rhs=bo_r[:],
                        start=False, stop=True,
                    )
                    ot = evac.tile([128, 512], f32, tag="evac")
                    nc.vector.tensor_copy(ot[:], po[:, half, :])
                    nc.scalar.dma_start(out_view[:, qc, :], ot[:])

        # Q^T pair-packed [128, 4, R]
        QT = qt_p.tile([128, 4, R], bf16)
        own_proj_packed(QT, w_qkv["q"], bqs2)


        # ---- attention machinery (pass = pair of heads) --------------
        OT = ot_p.tile([128, 4, R], bf16)  # outH^T normalized, pair-packed

        # Each pass's two AV accumulators live in the halves of ONE 2-bank
        # "big" psum tile, so the 1-bank "po" tag stays free for transposes
        # and broadcasts, and consecutive passes can overlap their flushes.
        def emit_attn_chunk(pc, ch, acc, pend):
            h0, h1 = 2 * pc, 2 * pc + 1
            pl = ps_big.tile([128, 2, 512], f32, tag="big")
            nc.tensor.matmul(
                pl[:, 0, :],
                lhsT=KT[0:64, pc, ch * 128:(ch + 1) * 128],
                rhs=QT[0:64, pc, :], start=True, stop=True,
            )
            nc.tensor.matmul(
                pl[:, 1, :],
                lhsT=KT[64:128, pc, ch * 128:(ch + 1) * 128],
                rhs=QT[64:128, pc, :], start=True, stop=True,
            )
            pexp = pexp_p.tile([128, 2, 512], bf16, tag="pexp")
            nc.scalar.activation(pexp[:], pl[:], AF.Exp, scale=SCALE)
            if pend[0] is not None:
                ppexp, pch = pend[0]
                nc.tensor.matmul(
                    acc[0:E + 1, 0, :], lhsT=VP[:, pch, h0, :],
                    rhs=ppexp[:, 0, :], start=(pch == 0), stop=False,
                )
                nc.tensor.matmul(
                    acc[0:E + 1, 1, :], lhsT=VP[:, pch, h1, :],
                    rhs=ppexp[:, 1, :], start=(pch == 0), stop=False,
                )
            pend[0] = (pexp, ch)

        def flush_attn_pass(pc, acc, pend):
            h0, h1 = 2 * pc, 2 * pc + 1
            ppexp, pch = pend[0]
            nc.tensor.matmul(
                acc[0:E + 1, 0, :], lhsT=VP[:, pch, h0, :],
                rhs=ppexp[:, 0, :], start=False, stop=True,
            )
            nc.tensor.matmul(
                acc[0:E + 1, 1, :], lhsT=VP[:, pch, h1, :],
                rhs=ppexp[:, 1, :], start=False, stop=True,
            )
            # normalize rows 0..63 by the ones-column row 64; h1 halves are
            # moved to partitions 64:128 by an SBUF->SBUF DMA (pair packing).
            # The accumulator evacuation runs on DVE so its psum frees
            # without queuing behind the exp backlog on ACT.
            for half, h in ((0, h0), (1, h1)):
                otr = otr_p.tile([E + 1, R], f32, tag="otr")
                nc.vector.tensor_copy(otr[:], acc[0:E + 1, half, :])
                rden = otr1.tile([1, R], bf16, tag="rden")
                nc.vector.reciprocal(rden[:], otr[E:E + 1, :])
                pb = ps_po.tile([E, R], f32, tag="po")
                nc.tensor.matmul(
                    pb[:], lhsT=ones_row[:, 0:E], rhs=rden[:],
                    start=True, stop=True,
                )
                if h == h0:
                    nc.vector.tensor_tensor(
                        OT[0:64, pc, :], otr[0:E, :], pb[:], ALU.mult
                    )
                else:
                    oth = otr1.tile([64, R], bf16, tag="oth")
                    nc.vector.tensor_tensor(oth[:], otr[0:E, :], pb[:], ALU.mult)
                    nc.gpsimd.dma_start(OT[64:128, pc, :], oth[:])

        # pass 0 is interleaved with phase 1 (its chunks consume K^T/V'
        # right as they are produced, soaking up phase-1 ACT slack)
        acc0 = ps_big.tile([128, 2, 512], f32, tag="big")
        pend0 = [None]

        for tt in range(NT):
            if tt in xin_pre:
                xin = xin_pre[tt]
            else:
                xin = xin_p.tile([128, 4, 512], f32, tag="xin")
                nc.sync.dma_start(xin[:], x_v[tt])
            xt = xt_p.tile([128, 4, 512], bf16, tag="xt")
            transpose_512(xt, xin)
            # K^T pair-packed: psum [128 he', 512 t] per mc
            for mch in range(2):
                pk = ps_big.tile([128, 2, 512], f32, tag="big")
                for half in range(2):
                    mc = mch * 2 + half
                    for dc in range(4):
                        nc.tensor.matmul(
                            pk[:, half, :],
                            lhsT=w_qkv["k"][:, dc, mc * 128:(mc + 1) * 128],
                            rhs=xt[:, dc, :],
                            start=(dc == 0), stop=(dc == 3),
                        )
                    nc.scalar.activation(
                        KT[:, mc, tt * 512:(tt + 1) * 512], pk[:, half, :],
                        AF.Identity, bias=bks2[:, mc:mc + 1],
                    )
            # V': psum [128 t, 512 he] per vc
            for vch in range(2):
                pv = ps_big.tile([128, 2, 512], f32, tag="big")
                for half in range(2):
                    vc = vch * 2 + half
                    for dc in range(4):
                        nc.tensor.matmul(
                            pv[:, half, :],
                            lhsT=xt[:, dc, vc * 128:(vc + 1) * 128],
                            rhs=w_qkv["v"][:, dc, :],
                            start=(dc == 0), stop=(dc == 3),
                        )
                    nc.vector.tensor_tensor(
                        VP[:, tt * 4 + vc, :, 0:E],
                        pv[:, half, :].rearrange("p (h e) -> p h e", e=E),
                        bv_bc[:].rearrange("p (h e) -> p h e", e=E),
                        ALU.add,
                    )
            if tt >= 1:
                # pass 0 runs one chunk-group behind phase 1, so its logits
                # never wait on this iteration's K^T/V' evacuations
                for cc in range(4):
                    emit_attn_chunk(0, (tt - 1) * 4 + cc, acc0, pend0)

        # W1/W2 loads go on the gpsimd queue here — early enough to stream
        # during attention, after the xt slots they reuse have freed
        W1_v = W1_d.rearrange("(dc p) f -> dc p f", p=128)
        W1_s = []
        for j in range(4):
            t = xt_p.tile([128, 4, 512], bf16, tag="xt")
            nc.gpsimd.dma_start(t[:].rearrange("p c q -> p (c q)"), W1_v[j])
            W1_s.append(t)
        W2_v = W2_d.rearrange("(g fc p) d -> g p fc d", p=128, fc=4)
        W2_s = []
        for j in range(4):
            t = xt_p.tile([128, 4, 512], bf16, tag="xt")
            nc.gpsimd.dma_start(t[:], W2_v[j])
            W2_s.append(t)

        # ---- phase 2: remaining attention passes ---------------------
        for c in range(28, 32):
            emit_attn_chunk(0, c, acc0, pend0)
        flush_attn_pass(0, acc0, pend0)
        for pc in range(1, 4):
            acc = ps_big.tile([128, 2, 512], f32, tag="big")
            pend = [None]
            for ch in range(32):
                emit_attn_chunk(pc, ch, acc, pend)
            flush_attn_pass(pc, acc, pend)
            if pc < 3:
                # keep the collective stream warm (a cold mesh AllReduce
                # costs ~30-50us, a warm one ~10us). The input is copied
                # from this pass's output so the trigger can't be hoisted
                # ahead of the pass by the scheduler.
                wt = wk.tile([1, 2], f32, tag="warm2")
                nc.vector.tensor_copy(wt[:], OT[0:1, pc, 0:2])
                cw_i = dram.tile([1, 2], f32)
                cw_o = dram.tile([1, 2], f32)
                nc.sync.dma_start(cw_i[:], wt[:])
                nc.gpsimd.collective_compute(
                    "AllReduce", ALU.add,
                    replica_groups=[list(range(N_CORES))],
                    ins=[cw_i[:]], outs=[cw_o[:]],
                )

        # ---- phase 3: out proj + residual + global LN1 ---------------
        z = xro  # in place: z = x + out
        for qch in range(2):
            po = ps_big.tile([128, 2, 512], f32, tag="big")
            for half in range(2):
                qc = qch * 2 + half
                for pc in range(4):
                    nc.tensor.matmul(
                        po[:, half, :],
                        lhsT=OT[:, pc, qc * 128:(qc + 1) * 128],
                        rhs=Wo_p[:, pc, :],
                        start=(pc == 0), stop=False,
                    )
                nc.tensor.matmul(
                    po[:, half, :], lhsT=ones_row[:], rhs=bo_r[:],
                    start=False, stop=True,
                )
                nc.vector.tensor_tensor(z[:, qc, :], po[:, half, :],
                                        xro[:, qc, :], ALU.add)

        def stats_start(src_t, tag):
            """Partial [sum, sumsq] -> AllReduce; returns output dram tile."""
            sums = wk.tile([128, 2], f32, tag=f"sums{tag}")
            nc.vector.tensor_reduce(
                out=sums[:, 0:1], in_=src_t[:], axis=AX.XY, op=ALU.add
            )
            sq = stg_p.tile([128, 4, D], f32, tag="stg")
            nc.scalar.activation(
                sq[:], src_t[:], AF.Square, accum_out=sums[:, 1:2]
            )
            pr = ps_po.tile([1, 2], f32, tag="po")
            nc.tensor.matmul(
                pr[:], lhsT=onesP[:, 0:1], rhs=sums[:], start=True, stop=True
            )
            part = wk.tile([1, 2], f32, tag=f"part{tag}")
            nc.vector.tensor_copy(part[:], pr[:])
            cin = dram.tile([1, 2], f32)
            cout = dram.tile([1, 2], f32)
            nc.sync.dma_start(cin[:], part[:])
            nc.gpsimd.collective_compute(
                "AllReduce", ALU.add,
                replica_groups=[list(range(N_CORES))],
                ins=[cin[:]], outs=[cout[:]],
            )
            return cout

        def stats_finish(cout, tag):
            """-> [128, 2] sbuf tile: [:,0]=rstd, [:,1]=-mu*rstd (global)."""
            tot = wk.tile([1, 2], f32, tag=f"tot{tag}")
            nc.sync.dma_start(tot[:], cout[:])
            sc = wk.tile([1, 6], f32, tag=f"sc{tag}")
            mu, m2 = sc[0:1, 0:1], sc[0:1, 1:2]
            nc.vector.tensor_scalar_mul(mu, tot[0:1, 0:1], INV_SD)
            nc.vector.tensor_scalar_mul(m2, tot[0:1, 1:2], INV_SD)
            nc.vector.tensor_tensor(sc[0:1, 2:3], mu, mu, ALU.mult)
            nc.vector.tensor_tensor(sc[0:1, 3:4], m2, sc[0:1, 2:3], ALU.subtract)
            nc.scalar.activation(sc[0:1, 4:5], sc[0:1, 3:4], AF.Sqrt, bias=eps_t[:])
            st2 = wk.tile([1, 2], f32r, tag=f"st2{tag}")
            nc.vector.reciprocal(st2[0:1, 0:1], sc[0:1, 4:5])        # rstd
            nc.vector.tensor_tensor(sc[0:1, 5:6], mu, st2[0:1, 0:1], ALU.mult)
            nc.vector.tensor_scalar_mul(st2[0:1, 1:2], sc[0:1, 5:6], -1.0)
            pbc = ps_po.tile([128, 2], f32, tag="po")
            nc.tensor.matmul(pbc[:], lhsT=ones_row_r[:], rhs=st2[:],
                             start=True, stop=True)
            stb = wk.tile([128, 2], f32, tag=f"stb{tag}")
            nc.vector.tensor_copy(stb[:], pbc[:])
            return stb

        def ln_apply(dst_tile, src_t, stb, store_view=None):
            for qc in range(4):
                n_t = evac.tile([128, D], f32, tag="evac")
                nc.scalar.activation(
                    n_t[:], src_t[:, qc, :], AF.Identity,
                    bias=stb[:, 1:2], scale=stb[:, 0:1],
                )
                nc.vector.tensor_tensor(n_t[:], n_t[:], lng_s[:, qc, :], ALU.mult)
                nc.vector.tensor_tensor(dst_tile[:, qc, :], n_t[:],
                                        lnb_s[:, qc, :], ALU.add)
                if store_view is not None:
                    nc.sync.dma_start(store_view[:, qc, :], dst_tile[:, qc, :])

        cout1 = stats_start(z, "a")
        # Kp + Vp own-projections fill the first AllReduce's latency window
        KTo = d16.tile([128, 4, R], bf16, tag="d16")
        own_proj_packed(KTo, w_qkv["k"], bks2)
        wo_project_packed(KTo, kp_v)
        VTo = qt_p.tile([128, 4, R], bf16, tag="qt")  # reuses QT slot
        own_proj_packed(VTo, w_qkv["v"], bvs2)
        stb1 = stats_finish(cout1, "a")
        out1 = c8.tile([128, 4, D], f32, tag="c8")
        ln_apply(out1, z, stb1)
        # out1^T bf16 via PE transpose (cast happens in the evacuation)
        out1T = single.tile([128, 4, R], bf16, tag="out1T")
        transpose_512(out1T, out1)

        # ---- phase 4: MLP + residual + global LN2 --------------------
        h1T = []
        for j in range(2):
            h1t_half = stg_p.tile([128, 8, R], bf16, tag="stg")
            h1T.append(h1t_half)
        for fmh in range(8):
            ph = ps_big.tile([128, 2, 512], f32, tag="big")
            for half in range(2):
                fm = fmh * 2 + half
                nc.tensor.matmul(
                    ph[:, half, :],
                    lhsT=W1_s[0][:].rearrange("p c q -> p (c q)")[
                        :, fm * 128:(fm + 1) * 128],
                    rhs=out1T[:, 0, :], start=True, stop=False,
                )
                for dc in range(1, 4):
                    nc.tensor.matmul(
                        ph[:, half, :],
                        lhsT=W1_s[dc][:].rearrange("p c q -> p (c q)")[
                            :, fm * 128:(fm + 1) * 128],
                        rhs=out1T[:, dc, :],
                        start=False, stop=(dc == 3),
                    )
                nc.scalar.activation(
                    h1T[fm // 8][:, fm % 8, :], ph[:, half, :], AF.Relu,
                    bias=b1s[:, fm:fm + 1],
                )
        # Vp output projection rides along with the MLP (PE slack) instead
        # of sitting after stats2 where it missed the AR2 window
        wo_project_packed(VTo, vp_v)

        w = out1  # in place: w = out1 + out2
        for qch in range(2):
            po = ps_big.tile([128, 2, 512], f32, tag="big")
            for half in range(2):
                qc = qch * 2 + half
                for fm in range(16):
                    nc.tensor.matmul(
                        po[:, half, :],
                        lhsT=h1T[fm // 8][:, fm % 8, qc * 128:(qc + 1) * 128],
                        rhs=W2_s[fm // 4][:, fm % 4, :],
                        start=(fm == 0), stop=False,
                    )
                nc.tensor.matmul(
                    po[:, half, :], lhsT=ones_row[:], rhs=b2_r[:],
                    start=False, stop=True,
                )
                nc.vector.tensor_tensor(w[:, qc, :], po[:, half, :],
                                        out1[:, qc, :], ALU.add)

        cout2 = stats_start(w, "b")
        stb2 = stats_finish(cout2, "b")
        fin_s = c8.tile([128, 4, D], f32, tag="c8")
        ln_apply(fin_s, w, stb2, store_view=fin_v)

    split_waits(nc)
    return nc


_NC_CACHE = None


def _get_nc():
    global _NC_CACHE
    if _NC_CACHE is None:
        _NC_CACHE = build_nc()
    return _NC_CACHE


def kernel(**inputs):
    inp = {k: np.ascontiguousarray(np.asarray(v, dtype=np.float32))
           for k, v in inputs.items()}
    in_maps = []
    for c in range(N_CORES):
        rows = slice(c * R, (c + 1) * R)
        in_maps.append(dict(
            x=inp["x"], Wq=inp["Wq"], Wk=inp["Wk"], Wv=inp["Wv"],
            bq=inp["bq"], bk=inp["bk"], bv=inp["bv"],
            Wo=inp["Wo"], bo=inp["bo"], W1=inp["W1"], b1=inp["b1"],
            W2=inp["W2"], b2=inp["b2"],
            x_rows=inp["x"][rows],
            ln_g_rows=inp["ln_g"][rows], ln_b_rows=inp["ln_b"][rows],
        ))
    nc = _get_nc()
    res = run_bass_kernel_spmd(nc, in_maps, list(range(N_CORES)))
    final = np.concatenate([res.results[c]["final_rows"] for c in range(N_CORES)])
    Kp = np.concatenate([res.results[c]["Kp_rows"] for c in range(N_CORES)])
    Vp = np.concatenate([res.results[c]["Vp_rows"] for c in range(N_CORES)])
    return (final, Kp, Vp)
